# revision 56
# baseline (speedup 1.0000x reference)
"""Trainium2 Bass kernel for MHA with RoPE (dense transformer block).

Problem shapes: h [1, 4096, 1024], 16 heads x 64 dim, full (non-causal)
softmax attention, post-softmax all-ones mask (identity), torch-Linear
projections with bias.

Sharding: head-parallel across 8 cores (2 heads/core).

Prologue (projections + chunk-0 scores):
  - a burst of M=1 warm-up matmuls on a memset tile (no DMA dependency)
    opens the PE HAM clock gate (1.2 -> 2.4 GHz) before real data lands.
  - host pre-arranges h / q/k/v weights partition-major so every input
    DMA is one contiguous descriptor per partition; h streams as chunk
    PAIRS (16KB descriptors) on the sync queue, chunk 0's second half +
    cos/sin ride the vector queue, weights scalar, consts gpsimd.
  - RoPE via PE rotation matmul: qs = R*(q+b) where R is the
    rotate-half permutation as an fp16 [128,128] matrix; then on DVE
    q' = (q+b) o cos + qs o sin (bias folded into ACT staging copy);
    the PE tails (rotation/v-transpose) are software-pipelined one
    projection group behind so the PE never waits on ACT staging.
  - v transposed to key-major via fp16 PE transpose; bias bv is folded
    into bo on the host (softmax rows sum to 1 exactly).
  - chunk 0's QK+exp run inside the prologue (one proj-chunk of lag)
    riding the otherwise idle ACT/DVE capacity.

Attention (PE column-streaming bound, ~950ns per key-tile iter at the
warm 2.4GHz clock -- ~4.3 N=512 matmuls/iter at the ~215ns issue floor):
  - per 512-query chunk x 32 key tiles: row-tiled QK pair (K=64 per
    head at tile_position (0,0)/(64,0), both heads concurrent, no
    zero-padding), one [128,1024] 2-bank psum score tile per iteration
    from a 3-slot ring shared with o_proj/broadcast borrows.
  - exp strictly alternates between ACT (table exp, scale=1/8) and DVE
    (Schraudolph: fp16 bits = int16(score*A + B), one tensor_scalar,
    bitcast free), halving the softmax wall time; rel err ~6.4e-3.
  - PV runs one full chunk behind QK/exp from a 40-deep pt ring, with
    a ones-row appended to v (denominator for free, M=65).
  - each chunk's epilogue (stage copy incl. den row -> Ln(den) ->
    rec=exp(-ln den + ln4096) -> K=1 ones broadcast matmul ->
    normalize) is sliced across the next loop's iterations so no piece
    blocks the in-order PE queue or delays a ring-critical exp; the
    split DVE+ACT stage copy alone releases the cx psum slot.  o_proj
    row-blocks + fp16 staging copies are likewise spread out.
  - the drain pipelines the last chunk's epilogue per 128-query block
    and bridges its serial ACT chain with throwaway matmuls so the HAM
    clock gate stays open.

Host sums the 8 partial outputs, divides by 4096, adds bo + wo@bv.
"""

import math

import numpy as np

HIDDEN = 1024
HEADS = 16
HEAD_DIM = 64
SEQ = 4096
NCORES = 8
FPC = 128  # features per core = 2 heads x 64

# Schraudolph fp16-exp constants: fp16bits(e^(s/8)) ~= int16(s*EXP_A + EXP_B)
EXP_A = 1024.0 * 0.125 / math.log(2.0)
EXP_B = 15360.0 - 44.0

_NC_CACHE = {}


def _build_nc(S=SEQ):
    import concourse.bass as bass
    import concourse.tile as tile
    from concourse import mybir
    from contextlib import ExitStack

    f32 = mybir.dt.float32
    f16 = mybir.dt.float16
    i16 = mybir.dt.int16
    Exp = mybir.ActivationFunctionType.Exp
    Ln = mybir.ActivationFunctionType.Ln
    Ident = mybir.ActivationFunctionType.Identity
    Mult = mybir.AluOpType.mult
    Add = mybir.AluOpType.add

    D = HEAD_DIM
    HID = HIDDEN
    KT = HID // 128  # hidden contraction tiles
    PC = 512         # projection seq chunk
    NPC = S // PC
    CH = 512         # attention query chunk
    NCH = S // CH
    SK = S // 128    # key tiles
    PVLAG = 4        # software-pipeline lag of PV behind QK/exp

    nc = bass.Bass(trn_type="TRN2")

    # host pre-arranges h and the q/k/v weights partition-major so every
    # input DMA is one contiguous descriptor per partition (the naive
    # "(ko p) s" rearrange yields 1024 scattered 256B descriptors and a
    # ~20us transfer)
    hR = nc.dram_tensor("hR", [128, NPC, KT, PC], f16, kind="ExternalInput")
    # q/k/v weights + rotation/identity matrices merged: one DMA of 128
    # fat (6.5KB) descriptors -- the DGE queues are descriptor-rate bound
    # (~33 descriptors/us), not bandwidth bound, so fewer+fatter
    # descriptors win; the tiny matrices ride along for free and keep the
    # slow gpsimd software DMA queue nearly empty
    wqkvR = nc.dram_tensor("wqkvR", [128, 26, 128], f16, kind="ExternalInput")
    bqk = nc.dram_tensor("bqk", [FPC, 2], f32, kind="ExternalInput")
    woT = nc.dram_tensor("woT", [FPC, HID], f16, kind="ExternalInput")
    # cos/sin packed per chunk on 64 partitions (rope multiplies read the
    # tables at a different partition base per head half, so no duplicate)
    csT = nc.dram_tensor("csT", [D, NPC, 2, PC], f16, kind="ExternalInput")
    out = nc.dram_tensor("out", [S, HID], f16, kind="ExternalOutput")



    with tile.TileContext(nc) as tc, ExitStack() as top:
        sing = top.enter_context(tc.tile_pool(name="sing", bufs=1))

        # HAM warm-up: the PE clock-gate only opens (1.2 -> 2.4 GHz) after
        # ~3.4us of sustained matmul activity, and input data doesn't land
        # until ~12us.  A burst of M=1 throwaway matmuls on a memset tile
        # (no DMA dependency) warms the gate so the first real projection
        # matmul already streams at full clock.
        wsrc = sing.tile([128, 512], f16)
        nc.gpsimd.memset(wsrc, 0.0)
        with tc.tile_pool(name="wup", bufs=1, space="PSUM") as wup:
            wps = wup.tile([1, 1024], f32)
            for j in range(36):
                nc.tensor.matmul(
                    wps[0:1, (j % 2) * 512 : (j % 2) * 512 + 512],
                    wsrc[:, 0:1],
                    wsrc[:, 0:512],
                    start=True,
                    stop=True,
                    skip_group_check=True,
                )

        # input DMAs are spread across the two hardware DGE queues (sync +
        # scalar) in need-order: h chunks own the sync queue; the scalar
        # queue carries wq, then chunk 0's cos/sin slice, then wk/wv, then
        # the remaining tables and wo; tiny consts ride gpsimd's software
        # queue.
        wqkv_sb = sing.tile([128, 26, 128], f16)
        nc.scalar.dma_start(wqkv_sb, wqkvR[:, :, :])
        wq_sb = wqkv_sb[:, 0:8]
        wk_sb = wqkv_sb[:, 8:16]
        wv_sb = wqkv_sb[:, 16:24]
        rot_sb = wqkv_sb[:, 24, :]
        ident_sb = wqkv_sb[:, 25, :]
        # tables live on all 128 partitions (engines can't read across
        # partition lanes); sliced need-order DMAs so chunk ch's slice
        # lands before its rope tail / QK-ahead runs
        cs_sb = sing.tile([128, NPC, 2, PC], f16)
        nc.scalar.dma_start(cs_sb[0:64, 0:2, :, :], csT[:, 0:2, :, :])
        nc.scalar.dma_start(cs_sb[64:128, 0:2, :, :], csT[:, 0:2, :, :])
        nc.scalar.dma_start(cs_sb[0:64, 2:5, :, :], csT[:, 2:5, :, :])
        nc.scalar.dma_start(cs_sb[64:128, 2:5, :, :], csT[:, 2:5, :, :])
        nc.scalar.dma_start(cs_sb[0:64, 5:, :, :], csT[:, 5:, :, :])
        nc.scalar.dma_start(cs_sb[64:128, 5:, :, :], csT[:, 5:, :, :])
        wo_sb = sing.tile([FPC, HID], f16)
        nc.scalar.dma_start(wo_sb, woT[:, :])
        b_sb = sing.tile([FPC, 2], f32)
        nc.gpsimd.dma_start(b_sb, bqk[:, :])
        ones_sb = sing.tile([1, 64], f16)
        nc.vector.memset(ones_sb, 1.0)
        ln4096_sb = sing.tile([1, 1], f32)
        nc.vector.memset(ln4096_sb, math.log(4096.0))

        # warm the natural_log_exp table set during the prologue so the
        # first attention exp doesn't eat the ~2.7us ACT_TABLE_LOAD
        warm_sb = sing.tile([1, 1], f32)
        nc.scalar.activation(warm_sb, ln4096_sb, Ln)
        nc.scalar.activation(warm_sb, warm_sb, Exp)

        qT_sb = sing.tile([128, S], f16)
        kp_sb = sing.tile([128, S], f16)
        # v1[:, hh, i, 0:64] = v tile (keys-major); [.., 64] = ones row so the
        # PV matmul also produces the softmax denominator.
        v1_sb = sing.tile([128, 2, SK, 65], f16)
        nc.gpsimd.memset(v1_sb, 1.0)
        ctx_sb = sing.tile([128, S], f16)

        # pt tiles span both phases: chunk 0's probabilities are computed
        # during the projection prologue (exp-ahead) and consumed by PV in
        # the attention phase; the ring is deep enough to hold one full
        # chunk (32) plus PVL lagged tiles and in-flight exps.
        ptp = top.enter_context(tc.tile_pool(name="ptp", bufs=38))

        def emit_qk_exp(pool, pts_list, c, i, eng=None):
            cs0 = c * CH
            ksl = slice(i * 128, (i + 1) * 128)
            csl = slice(cs0, cs0 + CH)
            ss = pool.tile([128, 2 * CH], f32, tag="ss", name=f"ss_{c}_{i}")
            for hh in range(2):
                psl = slice(hh * 64, hh * 64 + 64)
                nc.tensor.matmul(
                    ss[:, hh * CH : (hh + 1) * CH],
                    kp_sb[psl, ksl],
                    qT_sb[psl, csl],
                    start=True,
                    stop=True,
                )
            pt = ptp.tile([128, 2 * CH], f16, tag="pt", name=f"pt_{c}_{i}")
            # strict alternation: back-to-back same-engine exps serialize
            # and stall the ss ring
            if (eng if eng is not None else i) % 2 == 1:
                nc.scalar.activation(pt, ss, Exp, scale=0.125)
            else:
                nc.vector.tensor_scalar(
                    pt[:, :].bitcast(i16), ss, EXP_A, EXP_B, Mult, Add
                )
            pts_list[i] = pt

        pts0 = [None] * SK

        # ---- projections + RoPE + v transpose (+ chunk-0 QK/exp) ----
        with ExitStack() as ph1:
            hp = ph1.enter_context(tc.tile_pool(name="hp", bufs=2))
            rp = ph1.enter_context(tc.tile_pool(name="rope", bufs=8))
            pps = ph1.enter_context(tc.tile_pool(name="pps", bufs=2, space="PSUM"))
            qsp = ph1.enter_context(tc.tile_pool(name="qsp", bufs=1, space="PSUM"))
            tps = ph1.enter_context(tc.tile_pool(name="tps", bufs=1, space="PSUM"))
            ssP = ph1.enter_context(tc.tile_pool(name="ssP", bufs=2, space="PSUM"))

            # h arrives as chunk PAIRS (16KB contiguous per partition per
            # DMA descriptor -- descriptor-rate, ~33 descriptors/us/queue,
            # not bandwidth, limits the DGE queues; splitting a pair costs
            # 128 extra descriptors and delays everything behind it)
            # pair 0 arrives as two single-chunk DMAs: chunk 0's sem fires
            # ~4us earlier so the first projection group starts sooner
            hpair = [None] * (NPC // 2)
            hpair[0] = hp.tile([128, 2, KT, PC], f16, tag="hpair", name="hpair_0")
            nc.sync.dma_start(hpair[0][:, 0], hR[:, 0, :, :])
            nc.sync.dma_start(hpair[0][:, 1], hR[:, 1, :, :])
            # Software-pipelined: each projection group's PE epilogue (the
            # rotation matmul / v transposes, which wait on an ACT staging
            # copy) is emitted under the NEXT group's matmul stream so the
            # PE never stalls on ACT latency.
            pend = []

            def rope_tail(ch, wi, stg, dst):
                ssl = slice(ch * PC, (ch + 1) * PC)
                qs = qsp.tile([128, PC], f32, tag="qs", name=f"qs_{ch}_{wi}")
                nc.tensor.matmul(qs, rot_sb, stg, start=True, stop=True)
                cosv = cs_sb[:, ch, 0, :]
                sinv = cs_sb[:, ch, 1, :]
                t1 = rp.tile([128, PC], f16, tag="t1", name=f"t1_{ch}_{wi}")
                nc.vector.tensor_mul(t1, stg, cosv)
                t2 = rp.tile([128, PC], f16, tag="t2", name=f"t2_{ch}_{wi}")
                nc.vector.tensor_mul(t2, qs, sinv)
                nc.vector.tensor_add(dst[:, ssl], t1, t2)

            def v_tail(ch, stgv):
                for st in range(PC // 128):
                    kti = ch * (PC // 128) + st
                    tp = tps.tile([128, 128], f16, tag="tp", name=f"tp_{ch}_{st}")
                    nc.tensor.transpose(
                        tp, stgv[:, st * 128 : (st + 1) * 128], ident_sb
                    )
                    nc.vector.tensor_copy(v1_sb[:, :, kti, 0:64], tp)

            for ch in range(NPC):
                ssl = slice(ch * PC, (ch + 1) * PC)
                p, half = divmod(ch, 2)
                if half == 0 and ch >= 2:
                    hpair[p] = hp.tile(
                        [128, 2, KT, PC], f16, tag="hpair", name=f"hpair_{p}"
                    )
                    nc.sync.dma_start(hpair[p], hR[:, ch : ch + 2, :, :])
                h_sb = hpair[p][:, half]
                for wi, (w_sb, dst) in enumerate(
                    [(wq_sb, qT_sb), (wk_sb, kp_sb), (wv_sb, None)]
                ):
                    ps = pps.tile([128, PC], f32, tag="ps", name=f"ps_{ch}_{wi}")
                    for k in range(KT):
                        nc.tensor.matmul(
                            ps,
                            w_sb[:, k, :],
                            h_sb[:, k, :],
                            start=(k == 0),
                            stop=(k == KT - 1),
                        )
                    if pend:
                        pend.pop(0)()
                    if dst is not None:
                        # stage with bias on ACT (psum -> fp16 sbuf)
                        stg = rp.tile([128, PC], f16, tag="stg", name=f"stg_{ch}_{wi}")
                        nc.scalar.activation(
                            stg, ps, Ident, bias=b_sb[:, wi : wi + 1]
                        )
                        pend.append(
                            lambda ch=ch, wi=wi, stg=stg, dst=dst: rope_tail(
                                ch, wi, stg, dst
                            )
                        )
                    else:
                        stgv = rp.tile([128, PC], f16, tag="stgv", name=f"stgv_{ch}")
                        nc.scalar.activation(stgv, ps, Ident)
                        pend.append(lambda ch=ch, stgv=stgv: v_tail(ch, stgv))
                # chunk-0 attention scores for the key tiles of the PREVIOUS
                # proj chunk (one chunk of lag so the rope chain feeding kp is
                # long done and the in-order PE queue never waits on it); the
                # exp work rides the otherwise idle ACT/DVE prologue capacity
                if ch >= 1:
                    for i in range(4 * (ch - 1), 4 * ch):
                        emit_qk_exp(ssP, pts0, 0, i)
            # only the pipeline tails remain here (cheap PE work, no exp
            # gating); chunk-0's last 4 QK/exps are injected into chunk 1's
            # first attention iterations instead, so the PE flows from the
            # prologue into attention without an exp ping-pong that would
            # idle it >3.4us and HAM-re-throttle the clock.  The tails wait
            # on ACT stagings, leaving ~1us PE gaps -- pad with warmers so
            # the HAM activity window never reads idle here.
            def flush_warmer(tag, n=3):
                wf = ssP.tile([128, 2 * CH], f32, tag="ss", name=f"wfl_{tag}")
                for j in range(n):
                    nc.tensor.matmul(
                        wf[0:1, (j % 2) * CH : (j % 2) * CH + CH],
                        wsrc[:, 0:1],
                        wsrc[:, 0:CH],
                        start=True,
                        stop=True,
                        skip_group_check=True,
                    )

            flush_warmer("a")
            for w in pend:
                w()
                flush_warmer("b")

        # ---- attention + o_proj ----
        # PV runs one full chunk behind QK/exp: chunk c's loop emits QK/exp
        # for chunk c and PV for chunk c-1 from the persisted pt ring, so the
        # epilogue (den -> rec -> normalize) of each chunk has a whole chunk
        # of slack before its cx slot is needed again, and chunk 0's QK/exp
        # were already emitted in the prologue.
        with ExitStack() as ph2:
            ssp = ph2.enter_context(tc.tile_pool(name="ssp", bufs=3, space="PSUM"))
            cxp = ph2.enter_context(tc.tile_pool(name="cxp", bufs=1, space="PSUM"))
            obp = ph2.enter_context(tc.tile_pool(name="obp", bufs=6))
            epp = ph2.enter_context(tc.tile_pool(name="epp", bufs=2))

            def emit_oproj_pair(c, sq, final=False, dma=None):
                # one [128 q, 1024 hid] output row-block of chunk c's o_proj:
                # two matmuls into the two banks of one ss slot, one staging
                # copy, one DMA
                r0 = c * CH + sq * 128
                ops = ssp.tile([128, 2 * CH], f32, tag="ss", name=f"op_{c}_{sq}")
                for nz in range(2):
                    nc.tensor.matmul(
                        ops[:, nz * 512 : (nz + 1) * 512],
                        ctx_sb[:, r0 : r0 + 128],
                        wo_sb[:, nz * 512 : (nz + 1) * 512],
                        start=True,
                        stop=True,
                    )
                ob = obp.tile([128, 1024], f16, tag="ob", name=f"ob_{c}_{sq}")
                # staging split into halves so no single ACT/DVE queue
                # insertion overflows the slack between consecutive exps;
                # ACT takes most halves (DVE's exp is pricier)
                for nz in range(2):
                    osl = slice(nz * 512, (nz + 1) * 512)
                    # tail o_proj: both engines idle, strict alternation;
                    # steady state: 5:3 toward ACT (DVE's exp is pricier)
                    dve = (sq + nz) % 2 == 0 if final else (
                        nz == 0 and sq % 2 == 1
                    ) or (nz == 1 and sq == 2)
                    if dve:
                        nc.vector.tensor_copy(ob[:, osl], ops[:, osl])
                    else:
                        nc.scalar.activation(ob[:, osl], ops[:, osl], Ident)
                (dma or nc.sync).dma_start(out[r0 : r0 + 128, :], ob)

            def emit_pv(cx, pts_list, i):
                for hh in range(2):
                    nc.tensor.matmul(
                        cx[:, hh, :],
                        v1_sb[:, hh, i, :],
                        pts_list[i][:, hh * CH : (hh + 1) * CH],
                        start=(i == 0),
                        stop=(i == SK - 1),
                    )

            # The epilogue of chunk c-1 is sliced into small pieces spread
            # over the next loop's iterations so no piece blocks the in-order
            # PE queue at a boundary and no ACT/DVE insertion lands as one
            # big blob in front of a ring-critical exp:
            #   loop end: stage copy incl. den row, split DVE+ACT (releases
            #             cx by itself -- Ln reads the STAGED den, so the
            #             psum slot frees ~0.6us after the last PV)
            #   iter 1:   Ln(den) on ACT (2 per-head halves)
            #   iter 4:   rec = Exp(-ln den + ln 4096) on ACT
            #   iter 7:   rb broadcast (PE) + normalize muls (DVE)
            epi = {}

            def emit_stage(c, cx, dve_only=False):
                # dve_only: in the drain ACT is busy with ln/rec, DVE idle
                stage = epp.tile([65, 2, CH], f16, tag="stage", name=f"stage_{c}")
                nc.vector.tensor_copy(stage[:, 0, :], cx[:, 0, :])
                if dve_only:
                    nc.vector.tensor_copy(stage[:, 1, :], cx[:, 1, :])
                else:
                    nc.scalar.activation(stage[:, 1, :], cx[:, 1, :], Ident)
                epi.setdefault(c, {})["stage"] = stage

            def emit_ln(c, cx=None):
                # steady chunks read the STAGED den so the cx psum frees
                # early; the drain passes cx so ln runs concurrently with
                # the stage copy (cx release no longer matters there)
                lnb = epp.tile([1, 2 * CH], f32, tag="lnb", name=f"lnb_{c}")
                src = cx[64:65] if cx is not None else epi[c]["stage"][64:65]
                nc.scalar.activation(lnb[:, 0:CH], src[:, 0, :], Ln)
                nc.scalar.activation(lnb[:, CH : 2 * CH], src[:, 1, :], Ln)
                epi.setdefault(c, {})["lnb"] = lnb

            def emit_rec(c):
                rec = epp.tile([1, 2 * CH], f16, tag="rec", name=f"rec_{c}")
                # rec = 4096/den keeps fp16 in normal range; host undoes it
                nc.scalar.activation(
                    rec, epi[c]["lnb"], Exp, scale=-1.0, bias=ln4096_sb[:, :]
                )
                epi[c]["rec"] = rec

            def emit_rb(c):
                rb = ssp.tile([128, 2 * CH], f32, tag="ss", name=f"rb_{c}")
                for hh in range(2):
                    nc.tensor.matmul(
                        rb[0:64, hh * CH : (hh + 1) * CH],
                        ones_sb,
                        epi[c]["rec"][:, hh * CH : (hh + 1) * CH],
                        start=True,
                        stop=True,
                    )
                epi[c]["rb"] = rb

            def emit_norm(c, qsl):
                # normalize ctx columns qsl (relative to the chunk) on DVE
                cs0 = c * CH
                e = epi[c]
                for hh in range(2):
                    hsl = slice(hh * 64, hh * 64 + 64)
                    nc.vector.tensor_mul(
                        ctx_sb[hsl, cs0 + qsl.start : cs0 + qsl.stop],
                        e["stage"][0:64, hh, qsl],
                        e["rb"][0:64, hh * CH + qsl.start : hh * CH + qsl.stop],
                    )

            def emit_rbnorm(c):
                emit_rb(c)
                emit_norm(c, slice(0, CH))
                epi.pop(c)

            PVL = 4  # intra-loop PV lag: the previous chunk's split stage
            # copy releases the cx slot before the first lagged PV needs it

            def body(c, i, pts_cur, cx, pts_prev):
                if pts_cur is not None:
                    emit_qk_exp(ssp, pts_cur, c, i)
                if c == 1 and i < 4:
                    # chunk 0's leftover QK/exps (tiles 28-31), on the
                    # engine opposite this iteration's main exp
                    emit_qk_exp(ssp, pts0, 0, 28 + i, eng=i + 1)
                if i >= PVL:
                    emit_pv(cx, pts_prev, i - PVL)
                # epilogue pieces + o_proj for chunk c-2 (whose PVs finished
                # at the end of loop c-1; its stage copy is emitted between
                # iters 1 and 2 of THIS loop, so ln comes at i==3)
                if c >= 2:
                    if i == 3:
                        emit_ln(c - 2)
                    elif i == 5:
                        emit_rec(c - 2)
                    elif i == 8:
                        emit_rbnorm(c - 2)
                    elif i in (12, 16, 20, 24):
                        emit_oproj_pair(c - 2, (i - 12) // 4)

            # a few bridge warmers: the first attention QKs wait on
            # prologue psum-bank handoff; keep the PE ticking through the
            # transition so HAM doesn't re-throttle
            wt = ssp.tile([128, 2 * CH], f32, tag="ss", name="wtrans")
            for j in range(6):
                nc.tensor.matmul(
                    wt[0:1, (j % 2) * CH : (j % 2) * CH + CH],
                    wsrc[:, 0:1],
                    wsrc[:, 0:CH],
                    start=True,
                    stop=True,
                    skip_group_check=True,
                )

            pts_prev = pts0
            cx_prev = None
            for c in range(1, NCH):
                pts_cur = [None] * SK
                # the new chunk's first two QK/exps go on the engine queues
                # BEFORE the previous chunk's stage copy, so the ss ring
                # slots recycle without queuing behind epilogue inserts;
                # the cx alloc must follow the stage emission (ring bufs=1)
                body(c, 0, pts_cur, None, pts_prev)
                body(c, 1, pts_cur, None, pts_prev)
                if cx_prev is not None:
                    emit_stage(c - 2, cx_prev)
                cx = cxp.tile([65, 2, CH], f32, tag="cx", name=f"cx_{c - 1}")
                for i in range(2, SK):
                    body(c, i, pts_cur, cx, pts_prev)
                for i in range(SK - PVL, SK):
                    emit_pv(cx, pts_prev, i)
                cx_prev = cx
                pts_prev = pts_cur
            # drain: PV + epilogue + o_proj of the last two chunks.  The
            # final epilogue is pipelined per 128-query row block (norm ->
            # o_proj -> staging -> DMA) so the tail is a short pipeline, not
            # a serial chain; the last blocks' DMAs ride separate queues.
            emit_stage(NCH - 2, cx_prev)
            cx = cxp.tile([65, 2, CH], f32, tag="cx", name=f"cx_{NCH - 1}")
            # bridge the stage(NCH-2) latency: the drain has no QKs, so
            # its first PV would otherwise idle the PE ~1.5us
            wd = ssp.tile([128, 2 * CH], f32, tag="ss", name="wdrain0")
            for j in range(6):
                nc.tensor.matmul(
                    wd[0:1, (j % 2) * CH : (j % 2) * CH + CH],
                    wsrc[:, 0:1],
                    wsrc[:, 0:CH],
                    start=True,
                    stop=True,
                    skip_group_check=True,
                )
            for i in range(2, SK):
                body(NCH, i, None, cx, pts_prev)
            for i in range(SK - PVL, SK):
                emit_pv(cx, pts_prev, i)
            cl = NCH - 1
            # bridge the serial stage->ln->rec window with throwaway
            # matmuls so HAM doesn't re-throttle right before the final
            # o_proj blocks (idle >3.4us drops PE to 1.2GHz)
            wslot = ssp.tile([128, 2 * CH], f32, tag="ss", name="wdrain")
            for j in range(10):
                nc.tensor.matmul(
                    wslot[0:1, (j % 2) * CH : (j % 2) * CH + CH],
                    wsrc[:, 0:1],
                    wsrc[:, 0:CH],
                    start=True,
                    stop=True,
                    skip_group_check=True,
                )
            emit_ln(cl, cx=cx)
            emit_rec(cl)
            emit_stage(cl, cx, dve_only=True)
            emit_rb(cl)
            # norm3 must be emitted before op2: the "ss" ring has 3 slots,
            # so op2 reuses rb's bank -- every rb reader must precede it
            emit_norm(cl, slice(0, 128))
            emit_oproj_pair(cl, 0, final=True, dma=nc.sync)
            emit_norm(cl, slice(128, 256))
            emit_oproj_pair(cl, 1, final=True, dma=nc.scalar)
            emit_norm(cl, slice(256, 384))
            emit_norm(cl, slice(384, 512))
            emit_oproj_pair(cl, 2, final=True, dma=nc.sync)
            emit_oproj_pair(cl, 3, final=True, dma=nc.scalar)
            epi.pop(cl)
    return nc


def _legalize_sync_waits(nc, max_waits=1):
    """Cap sync waits per instruction for this container's walrus build.

    The bundled walrus encodes a limited number of sync-wait commands per
    instruction ("Too many sync wait commands" codegen error), while Tile
    attaches one wait per logical processor where needed. An attached wait
    is equivalent to a standalone preceding wait on the same engine (that
    is exactly what raw-bass `wait_ge` emits: a pure-wait
    InstEventSemaphore), so hoist the excess waits onto EventSemaphore
    instructions inserted right before the offender.
    """
    from concourse import mybir

    n_fixed = 0
    for fn in nc.m.functions:
        for b in fn.blocks:
            insts = b.instructions
            idx = 0
            while idx < len(insts):
                inst = insts[idx]
                si = inst.sync_info
                waits = list(si.on_wait) if si and si.on_wait else []
                if len(waits) > max_waits:
                    updates = list(si.on_update) if si and si.on_update else []
                    pre, keep = waits[: -max_waits], waits[-max_waits:]
                    clones = []
                    for j, w in enumerate(pre):
                        clones.append(
                            mybir.InstEventSemaphore(
                                name=f"{inst.name}_sw{j}",
                                engine=inst.engine,
                                ins=[],
                                outs=[],
                                sync_info=mybir.SyncInfo(on_wait=[w], on_update=[]),
                            )
                        )
                    inst.sync_info = mybir.SyncInfo(on_wait=keep, on_update=updates)
                    for j, clone in enumerate(clones):
                        insts.insert(idx + j, clone)
                        try:
                            nc.inst_map[clone.name] = clone
                        except Exception:
                            pass
                    idx += len(clones)
                    n_fixed += 1
                idx += 1
    return n_fixed


MM_DT = "float16"


def get_nc(S=SEQ, mm_dt=MM_DT):
    key = S
    if key not in _NC_CACHE:
        nc = _build_nc(S)
        _legalize_sync_waits(nc)
        _NC_CACHE[key] = nc
    return _NC_CACHE[key]


def make_in_maps(h, cos, sin, wq, bq, wk, bk, wv, bv, wo):
    """Host-side shard prep. h [B,S,HID] -> per-core input dict."""
    f16 = np.float16
    h = np.asarray(h, dtype=np.float32)
    S = h.shape[1]
    PC, KT = 512, HIDDEN // 128
    NPC = S // PC
    # hR[p, ch, ko, s'] = h[ch*PC+s', ko*128+p]: one contiguous 8KB
    # descriptor per partition per chunk DMA
    hR = np.ascontiguousarray(
        h[0].reshape(NPC, PC, KT, 128).transpose(3, 0, 2, 1).astype(f16)
    )
    # csT[d, ch, 0, s'] = cos[ch*PC+s', d]; [.., 1, .] = sin (64 partitions)
    cosT = np.asarray(cos, np.float32).T.reshape(HEAD_DIM, NPC, PC)
    sinT = np.asarray(sin, np.float32).T.reshape(HEAD_DIM, NPC, PC)
    csT = np.ascontiguousarray(
        np.stack([cosT, sinT], axis=2).astype(f16)
    )
    wq = np.asarray(wq, dtype=np.float32)
    wk = np.asarray(wk, dtype=np.float32)
    wv = np.asarray(wv, dtype=np.float32)
    wo = np.asarray(wo, dtype=np.float32)
    bq = np.asarray(bq, dtype=np.float32)
    bk = np.asarray(bk, dtype=np.float32)

    def wR(w, fs):
        # wR[p, ko, f] = w[fs][f, ko*128+p]
        return np.ascontiguousarray(
            w[fs, :].T.reshape(KT, 128, FPC).transpose(1, 0, 2).astype(f16)
        )

    # rotate-half as a matmul: qs = R @ q with R = blockdiag([[0,-I],[I,0]]).
    # matmul computes lhsT.T @ rhs so we feed R^T = blockdiag([[0,I],[-I,0]]).
    rotT_np = np.zeros((FPC, FPC), dtype=f16)
    for hh in range(2):
        o = hh * 64
        for j in range(32):
            rotT_np[o + 32 + j, o + j] = -1.0
            rotT_np[o + j, o + 32 + j] = 1.0
    ident_np = np.eye(128, dtype=f16)

    in_maps = []
    for c in range(NCORES):
        fs = slice(c * FPC, (c + 1) * FPC)
        in_maps.append(
            {
                "hR": hR,
                "wqkvR": np.ascontiguousarray(
                    np.concatenate(
                        [
                            np.stack(
                                [wR(wq, fs), wR(wk, fs), wR(wv, fs)], axis=1
                            ).reshape(128, 3 * (HIDDEN // 128), FPC),
                            rotT_np[:, None, :],
                            ident_np[:, None, :],
                        ],
                        axis=1,
                    )
                ),
                "bqk": np.ascontiguousarray(
                    np.stack([bq[fs], bk[fs]], axis=1).astype(np.float32)
                ),
                "woT": np.ascontiguousarray(wo[:, fs].T).astype(f16),
                "csT": csT,
            }
        )
    return in_maps


def kernel(h, mask, cos, sin, wq, bq, wk, bk, wv, bv, wo, bo, **_unused):
    # mask is all-ones per the problem spec; post-softmax where(mask==0) is a no-op.
    from concourse.bass_utils import run_bass_kernel_spmd

    h = np.asarray(h, dtype=np.float32)
    S = h.shape[1]
    nc = get_nc(S)
    in_maps = make_in_maps(h, cos, sin, wq, bq, wk, bk, wv, bv, wo)
    res = run_bass_kernel_spmd(nc, in_maps, core_ids=list(range(NCORES)))
    acc = np.zeros((S, HIDDEN), dtype=np.float64)
    for r in res.results:
        acc += r["out"].astype(np.float64)
    acc /= 4096.0
    bo_eff = np.asarray(bo, np.float64) + np.asarray(wo, np.float64) @ np.asarray(
        bv, np.float64
    )
    acc += bo_eff[None, :]
    return acc[None].astype(np.float32)



# revision 57
# speedup vs baseline: 1.0123x; 1.0123x over previous
"""Trainium2 Bass kernel for MHA with RoPE (dense transformer block).

Problem shapes: h [1, 4096, 1024], 16 heads x 64 dim, full (non-causal)
softmax attention, post-softmax all-ones mask (identity), torch-Linear
projections with bias.

Sharding: head-parallel across 8 cores (2 heads/core).

Prologue (projections + chunk-0 scores):
  - a burst of M=1 warm-up matmuls on a memset tile (no DMA dependency)
    opens the PE HAM clock gate (1.2 -> 2.4 GHz) before real data lands.
  - host pre-arranges h / q/k/v weights partition-major so every input
    DMA is one contiguous descriptor per partition; h streams as chunk
    PAIRS (16KB descriptors) on the sync queue, chunk 0's second half +
    cos/sin ride the vector queue, weights scalar, consts gpsimd.
  - RoPE via PE rotation matmul: qs = R*(q+b) where R is the
    rotate-half permutation as an fp16 [128,128] matrix; then on DVE
    q' = (q+b) o cos + qs o sin (bias folded into ACT staging copy);
    the PE tails (rotation/v-transpose) are software-pipelined one
    projection group behind so the PE never waits on ACT staging.
  - v transposed to key-major via fp16 PE transpose; bias bv is folded
    into bo on the host (softmax rows sum to 1 exactly).
  - chunk 0's QK+exp run inside the prologue (one proj-chunk of lag)
    riding the otherwise idle ACT/DVE capacity.

Attention (PE column-streaming bound, ~950ns per key-tile iter at the
warm 2.4GHz clock -- ~4.3 N=512 matmuls/iter at the ~215ns issue floor):
  - per 512-query chunk x 32 key tiles: row-tiled QK pair (K=64 per
    head at tile_position (0,0)/(64,0), both heads concurrent, no
    zero-padding), one [128,1024] 2-bank psum score tile per iteration
    from a 3-slot ring shared with o_proj/broadcast borrows.
  - exp strictly alternates between ACT (table exp, scale=1/8) and DVE
    (Schraudolph: fp16 bits = int16(score*A + B), one tensor_scalar,
    bitcast free), halving the softmax wall time; rel err ~6.4e-3.
  - PV runs one full chunk behind QK/exp from a 40-deep pt ring, with
    a ones-row appended to v (denominator for free, M=65).
  - each chunk's epilogue (stage copy incl. den row -> Ln(den) ->
    rec=exp(-ln den + ln4096) -> K=1 ones broadcast matmul ->
    normalize) is sliced across the next loop's iterations so no piece
    blocks the in-order PE queue or delays a ring-critical exp; the
    split DVE+ACT stage copy alone releases the cx psum slot.  o_proj
    row-blocks + fp16 staging copies are likewise spread out.
  - the drain pipelines the last chunk's epilogue per 128-query block
    and bridges its serial ACT chain with throwaway matmuls so the HAM
    clock gate stays open.

Host sums the 8 partial outputs, divides by 4096, adds bo + wo@bv.
"""

import math

import numpy as np

HIDDEN = 1024
HEADS = 16
HEAD_DIM = 64
SEQ = 4096
NCORES = 8
FPC = 128  # features per core = 2 heads x 64

# Schraudolph fp16-exp constants: fp16bits(e^(s/8)) ~= int16(s*EXP_A + EXP_B)
EXP_A = 1024.0 * 0.125 / math.log(2.0)
EXP_B = 15360.0 - 44.0

_NC_CACHE = {}


def _build_nc(S=SEQ):
    import concourse.bass as bass
    import concourse.tile as tile
    from concourse import mybir
    from contextlib import ExitStack

    f32 = mybir.dt.float32
    f16 = mybir.dt.float16
    i16 = mybir.dt.int16
    Exp = mybir.ActivationFunctionType.Exp
    Ln = mybir.ActivationFunctionType.Ln
    Ident = mybir.ActivationFunctionType.Identity
    Mult = mybir.AluOpType.mult
    Add = mybir.AluOpType.add

    D = HEAD_DIM
    HID = HIDDEN
    KT = HID // 128  # hidden contraction tiles
    PC = 512         # projection seq chunk
    NPC = S // PC
    CH = 512         # attention query chunk
    NCH = S // CH
    SK = S // 128    # key tiles
    PVLAG = 4        # software-pipeline lag of PV behind QK/exp

    nc = bass.Bass(trn_type="TRN2")

    # host pre-arranges h and the q/k/v weights partition-major so every
    # input DMA is one contiguous descriptor per partition (the naive
    # "(ko p) s" rearrange yields 1024 scattered 256B descriptors and a
    # ~20us transfer)
    hR = nc.dram_tensor("hR", [128, NPC, KT, PC], f16, kind="ExternalInput")
    # q/k/v weights + rotation/identity matrices merged: one DMA of 128
    # fat (6.5KB) descriptors -- the DGE queues are descriptor-rate bound
    # (~33 descriptors/us), not bandwidth bound, so fewer+fatter
    # descriptors win; the tiny matrices ride along for free and keep the
    # slow gpsimd software DMA queue nearly empty
    wqkvR = nc.dram_tensor("wqkvR", [128, 26, 128], f16, kind="ExternalInput")
    bqk = nc.dram_tensor("bqk", [FPC, 2], f32, kind="ExternalInput")
    woT = nc.dram_tensor("woT", [FPC, HID], f16, kind="ExternalInput")
    # cos/sin packed per chunk on 64 partitions (rope multiplies read the
    # tables at a different partition base per head half, so no duplicate)
    csT = nc.dram_tensor("csT", [D, NPC, 2, PC], f16, kind="ExternalInput")
    out = nc.dram_tensor("out", [S, HID], f16, kind="ExternalOutput")



    with tile.TileContext(nc) as tc, ExitStack() as top:
        sing = top.enter_context(tc.tile_pool(name="sing", bufs=1))

        # HAM warm-up: the PE clock-gate only opens (1.2 -> 2.4 GHz) after
        # ~3.4us of sustained matmul activity, and input data doesn't land
        # until ~12us.  A burst of M=1 throwaway matmuls on a memset tile
        # (no DMA dependency) warms the gate so the first real projection
        # matmul already streams at full clock.
        wsrc = sing.tile([128, 512], f16)
        nc.gpsimd.memset(wsrc, 0.0)
        with tc.tile_pool(name="wup", bufs=1, space="PSUM") as wup:
            wps = wup.tile([1, 1024], f32)
            for j in range(36):
                nc.tensor.matmul(
                    wps[0:1, (j % 2) * 512 : (j % 2) * 512 + 512],
                    wsrc[:, 0:1],
                    wsrc[:, 0:512],
                    start=True,
                    stop=True,
                    skip_group_check=True,
                )

        # input DMAs are spread across the two hardware DGE queues (sync +
        # scalar) in need-order: h chunks own the sync queue; the scalar
        # queue carries wq, then chunk 0's cos/sin slice, then wk/wv, then
        # the remaining tables and wo; tiny consts ride gpsimd's software
        # queue.
        wqkv_sb = sing.tile([128, 26, 128], f16)
        nc.scalar.dma_start(wqkv_sb, wqkvR[:, :, :])
        wq_sb = wqkv_sb[:, 0:8]
        wk_sb = wqkv_sb[:, 8:16]
        wv_sb = wqkv_sb[:, 16:24]
        rot_sb = wqkv_sb[:, 24, :]
        ident_sb = wqkv_sb[:, 25, :]
        # tables live on all 128 partitions (engines can't read across
        # partition lanes); sliced need-order DMAs so chunk ch's slice
        # lands before its rope tail / QK-ahead runs
        cs_sb = sing.tile([128, NPC, 2, PC], f16)
        nc.scalar.dma_start(cs_sb[0:64, 0:2, :, :], csT[:, 0:2, :, :])
        nc.scalar.dma_start(cs_sb[64:128, 0:2, :, :], csT[:, 0:2, :, :])
        nc.scalar.dma_start(cs_sb[0:64, 2:5, :, :], csT[:, 2:5, :, :])
        nc.scalar.dma_start(cs_sb[64:128, 2:5, :, :], csT[:, 2:5, :, :])
        nc.scalar.dma_start(cs_sb[0:64, 5:, :, :], csT[:, 5:, :, :])
        nc.scalar.dma_start(cs_sb[64:128, 5:, :, :], csT[:, 5:, :, :])
        wo_sb = sing.tile([FPC, HID], f16)
        nc.scalar.dma_start(wo_sb, woT[:, :])
        b_sb = sing.tile([FPC, 2], f32)
        nc.gpsimd.dma_start(b_sb, bqk[:, :])
        ones_sb = sing.tile([1, 64], f16)
        nc.vector.memset(ones_sb, 1.0)
        ln4096_sb = sing.tile([1, 1], f32)
        nc.vector.memset(ln4096_sb, math.log(4096.0))

        # warm the natural_log_exp table set during the prologue so the
        # first attention exp doesn't eat the ~2.7us ACT_TABLE_LOAD
        warm_sb = sing.tile([1, 1], f32)
        nc.scalar.activation(warm_sb, ln4096_sb, Ln)
        nc.scalar.activation(warm_sb, warm_sb, Exp)

        qT_sb = sing.tile([128, S], f16)
        kp_sb = sing.tile([128, S], f16)
        # v1[:, hh, i, 0:64] = v tile (keys-major); [.., 64] = ones row so the
        # PV matmul also produces the softmax denominator.
        v1_sb = sing.tile([128, 2, SK, 65], f16)
        nc.gpsimd.memset(v1_sb, 1.0)
        ctx_sb = sing.tile([128, S], f16)

        # pt tiles span both phases: chunk 0's probabilities are computed
        # during the projection prologue (exp-ahead) and consumed by PV in
        # the attention phase; the ring is deep enough to hold one full
        # chunk (32) plus PVL lagged tiles and in-flight exps.
        ptp = top.enter_context(tc.tile_pool(name="ptp", bufs=38))

        def emit_qk_exp(pool, pts_list, c, i, eng=None):
            cs0 = c * CH
            ksl = slice(i * 128, (i + 1) * 128)
            csl = slice(cs0, cs0 + CH)
            ss = pool.tile([128, 2 * CH], f32, tag="ss", name=f"ss_{c}_{i}")
            for hh in range(2):
                psl = slice(hh * 64, hh * 64 + 64)
                nc.tensor.matmul(
                    ss[:, hh * CH : (hh + 1) * CH],
                    kp_sb[psl, ksl],
                    qT_sb[psl, csl],
                    start=True,
                    stop=True,
                )
            pt = ptp.tile([128, 2 * CH], f16, tag="pt", name=f"pt_{c}_{i}")
            # strict alternation: back-to-back same-engine exps serialize
            # and stall the ss ring
            if (eng if eng is not None else i) % 2 == 1:
                nc.scalar.activation(pt, ss, Exp, scale=0.125)
            else:
                nc.vector.tensor_scalar(
                    pt[:, :].bitcast(i16), ss, EXP_A, EXP_B, Mult, Add
                )
            pts_list[i] = pt

        pts0 = [None] * SK

        # ---- projections + RoPE + v transpose (+ chunk-0 QK/exp) ----
        with ExitStack() as ph1:
            hp = ph1.enter_context(tc.tile_pool(name="hp", bufs=2))
            rp = ph1.enter_context(tc.tile_pool(name="rope", bufs=8))
            pps = ph1.enter_context(tc.tile_pool(name="pps", bufs=2, space="PSUM"))
            qsp = ph1.enter_context(tc.tile_pool(name="qsp", bufs=1, space="PSUM"))
            tps = ph1.enter_context(tc.tile_pool(name="tps", bufs=1, space="PSUM"))
            ssP = ph1.enter_context(tc.tile_pool(name="ssP", bufs=2, space="PSUM"))

            # h arrives as chunk PAIRS (16KB contiguous per partition per
            # DMA descriptor -- descriptor-rate, ~33 descriptors/us/queue,
            # not bandwidth, limits the DGE queues; splitting a pair costs
            # 128 extra descriptors and delays everything behind it)
            # pair 0 arrives as two single-chunk DMAs: chunk 0's sem fires
            # ~4us earlier so the first projection group starts sooner
            hpair = [None] * (NPC // 2)
            hpair[0] = hp.tile([128, 2, KT, PC], f16, tag="hpair", name="hpair_0")
            nc.sync.dma_start(hpair[0][:, 0], hR[:, 0, :, :])
            nc.sync.dma_start(hpair[0][:, 1], hR[:, 1, :, :])
            # Software-pipelined: each projection group's PE epilogue (the
            # rotation matmul / v transposes, which wait on an ACT staging
            # copy) is emitted under the NEXT group's matmul stream so the
            # PE never stalls on ACT latency.
            pend = []

            def rope_tail(ch, wi, stg, dst):
                ssl = slice(ch * PC, (ch + 1) * PC)
                qs = qsp.tile([128, PC], f32, tag="qs", name=f"qs_{ch}_{wi}")
                nc.tensor.matmul(qs, rot_sb, stg, start=True, stop=True)
                cosv = cs_sb[:, ch, 0, :]
                sinv = cs_sb[:, ch, 1, :]
                t1 = rp.tile([128, PC], f16, tag="t1", name=f"t1_{ch}_{wi}")
                nc.vector.tensor_mul(t1, stg, cosv)
                t2 = rp.tile([128, PC], f16, tag="t2", name=f"t2_{ch}_{wi}")
                nc.vector.tensor_mul(t2, qs, sinv)
                nc.vector.tensor_add(dst[:, ssl], t1, t2)

            def v_tail(ch, stgv):
                for st in range(PC // 128):
                    kti = ch * (PC // 128) + st
                    tp = tps.tile([128, 128], f16, tag="tp", name=f"tp_{ch}_{st}")
                    nc.tensor.transpose(
                        tp, stgv[:, st * 128 : (st + 1) * 128], ident_sb
                    )
                    nc.vector.tensor_copy(v1_sb[:, :, kti, 0:64], tp)

            for ch in range(NPC):
                ssl = slice(ch * PC, (ch + 1) * PC)
                p, half = divmod(ch, 2)
                if half == 0 and ch >= 2:
                    hpair[p] = hp.tile(
                        [128, 2, KT, PC], f16, tag="hpair", name=f"hpair_{p}"
                    )
                    nc.sync.dma_start(hpair[p], hR[:, ch : ch + 2, :, :])
                h_sb = hpair[p][:, half]
                for wi, (w_sb, dst) in enumerate(
                    [(wq_sb, qT_sb), (wk_sb, kp_sb), (wv_sb, None)]
                ):
                    ps = pps.tile([128, PC], f32, tag="ps", name=f"ps_{ch}_{wi}")
                    for k in range(KT):
                        nc.tensor.matmul(
                            ps,
                            w_sb[:, k, :],
                            h_sb[:, k, :],
                            start=(k == 0),
                            stop=(k == KT - 1),
                        )
                    if pend:
                        pend.pop(0)()
                    if dst is not None:
                        # stage with bias on ACT (psum -> fp16 sbuf)
                        stg = rp.tile([128, PC], f16, tag="stg", name=f"stg_{ch}_{wi}")
                        nc.scalar.activation(
                            stg, ps, Ident, bias=b_sb[:, wi : wi + 1]
                        )
                        pend.append(
                            lambda ch=ch, wi=wi, stg=stg, dst=dst: rope_tail(
                                ch, wi, stg, dst
                            )
                        )
                    else:
                        stgv = rp.tile([128, PC], f16, tag="stgv", name=f"stgv_{ch}")
                        nc.scalar.activation(stgv, ps, Ident)
                        pend.append(lambda ch=ch, stgv=stgv: v_tail(ch, stgv))
                # chunk-0 attention scores for the key tiles of the PREVIOUS
                # proj chunk (one chunk of lag so the rope chain feeding kp is
                # long done and the in-order PE queue never waits on it); the
                # exp work rides the otherwise idle ACT/DVE prologue capacity
                if ch >= 1:
                    for i in range(4 * (ch - 1), 4 * ch):
                        emit_qk_exp(ssP, pts0, 0, i)
            # only the pipeline tails remain here (cheap PE work, no exp
            # gating); chunk-0's last 4 QK/exps are injected into chunk 1's
            # first attention iterations instead, so the PE flows from the
            # prologue into attention without an exp ping-pong that would
            # idle it >3.4us and HAM-re-throttle the clock
            for w in pend:
                w()

        # ---- attention + o_proj ----
        # PV runs one full chunk behind QK/exp: chunk c's loop emits QK/exp
        # for chunk c and PV for chunk c-1 from the persisted pt ring, so the
        # epilogue (den -> rec -> normalize) of each chunk has a whole chunk
        # of slack before its cx slot is needed again, and chunk 0's QK/exp
        # were already emitted in the prologue.
        with ExitStack() as ph2:
            ssp = ph2.enter_context(tc.tile_pool(name="ssp", bufs=3, space="PSUM"))
            cxp = ph2.enter_context(tc.tile_pool(name="cxp", bufs=1, space="PSUM"))
            obp = ph2.enter_context(tc.tile_pool(name="obp", bufs=6))
            epp = ph2.enter_context(tc.tile_pool(name="epp", bufs=2))

            def emit_oproj_pair(c, sq, final=False, dma=None):
                # one [128 q, 1024 hid] output row-block of chunk c's o_proj:
                # two matmuls into the two banks of one ss slot, one staging
                # copy, one DMA
                r0 = c * CH + sq * 128
                ops = ssp.tile([128, 2 * CH], f32, tag="ss", name=f"op_{c}_{sq}")
                for nz in range(2):
                    nc.tensor.matmul(
                        ops[:, nz * 512 : (nz + 1) * 512],
                        ctx_sb[:, r0 : r0 + 128],
                        wo_sb[:, nz * 512 : (nz + 1) * 512],
                        start=True,
                        stop=True,
                    )
                ob = obp.tile([128, 1024], f16, tag="ob", name=f"ob_{c}_{sq}")
                # staging split into halves so no single ACT/DVE queue
                # insertion overflows the slack between consecutive exps;
                # ACT takes most halves (DVE's exp is pricier)
                for nz in range(2):
                    osl = slice(nz * 512, (nz + 1) * 512)
                    # tail o_proj: both engines idle, strict alternation;
                    # steady state: 5:3 toward ACT (DVE's exp is pricier)
                    dve = (sq + nz) % 2 == 0 if final else (
                        nz == 0 and sq % 2 == 1
                    ) or (nz == 1 and sq == 2)
                    if dve:
                        nc.vector.tensor_copy(ob[:, osl], ops[:, osl])
                    else:
                        nc.scalar.activation(ob[:, osl], ops[:, osl], Ident)
                (dma or nc.sync).dma_start(out[r0 : r0 + 128, :], ob)

            def emit_pv(cx, pts_list, i):
                for hh in range(2):
                    nc.tensor.matmul(
                        cx[:, hh, :],
                        v1_sb[:, hh, i, :],
                        pts_list[i][:, hh * CH : (hh + 1) * CH],
                        start=(i == 0),
                        stop=(i == SK - 1),
                    )

            # The epilogue of chunk c-1 is sliced into small pieces spread
            # over the next loop's iterations so no piece blocks the in-order
            # PE queue at a boundary and no ACT/DVE insertion lands as one
            # big blob in front of a ring-critical exp:
            #   loop end: stage copy incl. den row, split DVE+ACT (releases
            #             cx by itself -- Ln reads the STAGED den, so the
            #             psum slot frees ~0.6us after the last PV)
            #   iter 1:   Ln(den) on ACT (2 per-head halves)
            #   iter 4:   rec = Exp(-ln den + ln 4096) on ACT
            #   iter 7:   rb broadcast (PE) + normalize muls (DVE)
            epi = {}

            def emit_stage(c, cx, dve_only=False):
                # dve_only: in the drain ACT is busy with ln/rec, DVE idle
                stage = epp.tile([65, 2, CH], f16, tag="stage", name=f"stage_{c}")
                nc.vector.tensor_copy(stage[:, 0, :], cx[:, 0, :])
                if dve_only:
                    nc.vector.tensor_copy(stage[:, 1, :], cx[:, 1, :])
                else:
                    nc.scalar.activation(stage[:, 1, :], cx[:, 1, :], Ident)
                epi.setdefault(c, {})["stage"] = stage

            def emit_ln(c, cx=None):
                # steady chunks read the STAGED den so the cx psum frees
                # early; the drain passes cx so ln runs concurrently with
                # the stage copy (cx release no longer matters there)
                lnb = epp.tile([1, 2 * CH], f32, tag="lnb", name=f"lnb_{c}")
                src = cx[64:65] if cx is not None else epi[c]["stage"][64:65]
                nc.scalar.activation(lnb[:, 0:CH], src[:, 0, :], Ln)
                nc.scalar.activation(lnb[:, CH : 2 * CH], src[:, 1, :], Ln)
                epi.setdefault(c, {})["lnb"] = lnb

            def emit_rec(c):
                rec = epp.tile([1, 2 * CH], f16, tag="rec", name=f"rec_{c}")
                # rec = 4096/den keeps fp16 in normal range; host undoes it
                nc.scalar.activation(
                    rec, epi[c]["lnb"], Exp, scale=-1.0, bias=ln4096_sb[:, :]
                )
                epi[c]["rec"] = rec

            def emit_rb(c):
                rb = ssp.tile([128, 2 * CH], f32, tag="ss", name=f"rb_{c}")
                for hh in range(2):
                    nc.tensor.matmul(
                        rb[0:64, hh * CH : (hh + 1) * CH],
                        ones_sb,
                        epi[c]["rec"][:, hh * CH : (hh + 1) * CH],
                        start=True,
                        stop=True,
                    )
                epi[c]["rb"] = rb

            def emit_norm(c, qsl):
                # normalize ctx columns qsl (relative to the chunk) on DVE
                cs0 = c * CH
                e = epi[c]
                for hh in range(2):
                    hsl = slice(hh * 64, hh * 64 + 64)
                    nc.vector.tensor_mul(
                        ctx_sb[hsl, cs0 + qsl.start : cs0 + qsl.stop],
                        e["stage"][0:64, hh, qsl],
                        e["rb"][0:64, hh * CH + qsl.start : hh * CH + qsl.stop],
                    )

            def emit_rbnorm(c):
                emit_rb(c)
                emit_norm(c, slice(0, CH))
                epi.pop(c)

            PVL = 4  # intra-loop PV lag: the previous chunk's split stage
            # copy releases the cx slot before the first lagged PV needs it

            def body(c, i, pts_cur, cx, pts_prev):
                if pts_cur is not None:
                    emit_qk_exp(ssp, pts_cur, c, i)
                if c == 1 and i < 4:
                    # chunk 0's leftover QK/exps (tiles 28-31), on the
                    # engine opposite this iteration's main exp
                    emit_qk_exp(ssp, pts0, 0, 28 + i, eng=i + 1)
                if i >= PVL:
                    emit_pv(cx, pts_prev, i - PVL)
                # epilogue pieces + o_proj for chunk c-2 (whose PVs finished
                # at the end of loop c-1; its stage copy is emitted between
                # iters 1 and 2 of THIS loop, so ln comes at i==3)
                if c >= 2:
                    if i == 3:
                        emit_ln(c - 2)
                    elif i == 5:
                        emit_rec(c - 2)
                    elif i == 8:
                        emit_rbnorm(c - 2)
                    elif i in (12, 16, 20, 24):
                        emit_oproj_pair(c - 2, (i - 12) // 4)

            # a few bridge warmers: the first attention QKs wait on
            # prologue psum-bank handoff; keep the PE ticking through the
            # transition so HAM doesn't re-throttle
            wt = ssp.tile([128, 2 * CH], f32, tag="ss", name="wtrans")
            for j in range(6):
                nc.tensor.matmul(
                    wt[0:1, (j % 2) * CH : (j % 2) * CH + CH],
                    wsrc[:, 0:1],
                    wsrc[:, 0:CH],
                    start=True,
                    stop=True,
                    skip_group_check=True,
                )

            pts_prev = pts0
            cx_prev = None
            for c in range(1, NCH):
                pts_cur = [None] * SK
                # the new chunk's first two QK/exps go on the engine queues
                # BEFORE the previous chunk's stage copy, so the ss ring
                # slots recycle without queuing behind epilogue inserts;
                # the cx alloc must follow the stage emission (ring bufs=1)
                body(c, 0, pts_cur, None, pts_prev)
                body(c, 1, pts_cur, None, pts_prev)
                if cx_prev is not None:
                    emit_stage(c - 2, cx_prev)
                cx = cxp.tile([65, 2, CH], f32, tag="cx", name=f"cx_{c - 1}")
                for i in range(2, SK):
                    body(c, i, pts_cur, cx, pts_prev)
                for i in range(SK - PVL, SK):
                    emit_pv(cx, pts_prev, i)
                cx_prev = cx
                pts_prev = pts_cur
            # drain: PV + epilogue + o_proj of the last two chunks.  The
            # final epilogue is pipelined per 128-query row block (norm ->
            # o_proj -> staging -> DMA) so the tail is a short pipeline, not
            # a serial chain; the last blocks' DMAs ride separate queues.
            emit_stage(NCH - 2, cx_prev)
            cx = cxp.tile([65, 2, CH], f32, tag="cx", name=f"cx_{NCH - 1}")
            # bridge the stage(NCH-2) latency: the drain has no QKs, so
            # its first PV would otherwise idle the PE ~1.5us
            wd = ssp.tile([128, 2 * CH], f32, tag="ss", name="wdrain0")
            for j in range(6):
                nc.tensor.matmul(
                    wd[0:1, (j % 2) * CH : (j % 2) * CH + CH],
                    wsrc[:, 0:1],
                    wsrc[:, 0:CH],
                    start=True,
                    stop=True,
                    skip_group_check=True,
                )
            for i in range(2, SK):
                body(NCH, i, None, cx, pts_prev)
            for i in range(SK - PVL, SK):
                emit_pv(cx, pts_prev, i)
            cl = NCH - 1
            # bridge the serial stage->ln->rec window with throwaway
            # matmuls so HAM doesn't re-throttle right before the final
            # o_proj blocks (idle >3.4us drops PE to 1.2GHz)
            wslot = ssp.tile([128, 2 * CH], f32, tag="ss", name="wdrain")
            for j in range(10):
                nc.tensor.matmul(
                    wslot[0:1, (j % 2) * CH : (j % 2) * CH + CH],
                    wsrc[:, 0:1],
                    wsrc[:, 0:CH],
                    start=True,
                    stop=True,
                    skip_group_check=True,
                )
            emit_ln(cl, cx=cx)
            emit_rec(cl)
            emit_stage(cl, cx, dve_only=True)
            emit_rb(cl)
            # norm3 must be emitted before op2: the "ss" ring has 3 slots,
            # so op2 reuses rb's bank -- every rb reader must precede it
            emit_norm(cl, slice(0, 128))
            emit_oproj_pair(cl, 0, final=True, dma=nc.sync)
            emit_norm(cl, slice(128, 256))
            emit_oproj_pair(cl, 1, final=True, dma=nc.scalar)
            emit_norm(cl, slice(256, 384))
            emit_norm(cl, slice(384, 512))
            emit_oproj_pair(cl, 2, final=True, dma=nc.sync)
            emit_oproj_pair(cl, 3, final=True, dma=nc.scalar)
            epi.pop(cl)
    return nc


def _legalize_sync_waits(nc, max_waits=1):
    """Cap sync waits per instruction for this container's walrus build.

    The bundled walrus encodes a limited number of sync-wait commands per
    instruction ("Too many sync wait commands" codegen error), while Tile
    attaches one wait per logical processor where needed. An attached wait
    is equivalent to a standalone preceding wait on the same engine (that
    is exactly what raw-bass `wait_ge` emits: a pure-wait
    InstEventSemaphore), so hoist the excess waits onto EventSemaphore
    instructions inserted right before the offender.
    """
    from concourse import mybir

    n_fixed = 0
    for fn in nc.m.functions:
        for b in fn.blocks:
            insts = b.instructions
            idx = 0
            while idx < len(insts):
                inst = insts[idx]
                si = inst.sync_info
                waits = list(si.on_wait) if si and si.on_wait else []
                if len(waits) > max_waits:
                    updates = list(si.on_update) if si and si.on_update else []
                    pre, keep = waits[: -max_waits], waits[-max_waits:]
                    clones = []
                    for j, w in enumerate(pre):
                        clones.append(
                            mybir.InstEventSemaphore(
                                name=f"{inst.name}_sw{j}",
                                engine=inst.engine,
                                ins=[],
                                outs=[],
                                sync_info=mybir.SyncInfo(on_wait=[w], on_update=[]),
                            )
                        )
                    inst.sync_info = mybir.SyncInfo(on_wait=keep, on_update=updates)
                    for j, clone in enumerate(clones):
                        insts.insert(idx + j, clone)
                        try:
                            nc.inst_map[clone.name] = clone
                        except Exception:
                            pass
                    idx += len(clones)
                    n_fixed += 1
                idx += 1
    return n_fixed


MM_DT = "float16"


def get_nc(S=SEQ, mm_dt=MM_DT):
    key = S
    if key not in _NC_CACHE:
        nc = _build_nc(S)
        _legalize_sync_waits(nc)
        _NC_CACHE[key] = nc
    return _NC_CACHE[key]


def make_in_maps(h, cos, sin, wq, bq, wk, bk, wv, bv, wo):
    """Host-side shard prep. h [B,S,HID] -> per-core input dict."""
    f16 = np.float16
    h = np.asarray(h, dtype=np.float32)
    S = h.shape[1]
    PC, KT = 512, HIDDEN // 128
    NPC = S // PC
    # hR[p, ch, ko, s'] = h[ch*PC+s', ko*128+p]: one contiguous 8KB
    # descriptor per partition per chunk DMA
    hR = np.ascontiguousarray(
        h[0].reshape(NPC, PC, KT, 128).transpose(3, 0, 2, 1).astype(f16)
    )
    # csT[d, ch, 0, s'] = cos[ch*PC+s', d]; [.., 1, .] = sin (64 partitions)
    cosT = np.asarray(cos, np.float32).T.reshape(HEAD_DIM, NPC, PC)
    sinT = np.asarray(sin, np.float32).T.reshape(HEAD_DIM, NPC, PC)
    csT = np.ascontiguousarray(
        np.stack([cosT, sinT], axis=2).astype(f16)
    )
    wq = np.asarray(wq, dtype=np.float32)
    wk = np.asarray(wk, dtype=np.float32)
    wv = np.asarray(wv, dtype=np.float32)
    wo = np.asarray(wo, dtype=np.float32)
    bq = np.asarray(bq, dtype=np.float32)
    bk = np.asarray(bk, dtype=np.float32)

    def wR(w, fs):
        # wR[p, ko, f] = w[fs][f, ko*128+p]
        return np.ascontiguousarray(
            w[fs, :].T.reshape(KT, 128, FPC).transpose(1, 0, 2).astype(f16)
        )

    # rotate-half as a matmul: qs = R @ q with R = blockdiag([[0,-I],[I,0]]).
    # matmul computes lhsT.T @ rhs so we feed R^T = blockdiag([[0,I],[-I,0]]).
    rotT_np = np.zeros((FPC, FPC), dtype=f16)
    for hh in range(2):
        o = hh * 64
        for j in range(32):
            rotT_np[o + 32 + j, o + j] = -1.0
            rotT_np[o + j, o + 32 + j] = 1.0
    ident_np = np.eye(128, dtype=f16)

    in_maps = []
    for c in range(NCORES):
        fs = slice(c * FPC, (c + 1) * FPC)
        in_maps.append(
            {
                "hR": hR,
                "wqkvR": np.ascontiguousarray(
                    np.concatenate(
                        [
                            np.stack(
                                [wR(wq, fs), wR(wk, fs), wR(wv, fs)], axis=1
                            ).reshape(128, 3 * (HIDDEN // 128), FPC),
                            rotT_np[:, None, :],
                            ident_np[:, None, :],
                        ],
                        axis=1,
                    )
                ),
                "bqk": np.ascontiguousarray(
                    np.stack([bq[fs], bk[fs]], axis=1).astype(np.float32)
                ),
                "woT": np.ascontiguousarray(wo[:, fs].T).astype(f16),
                "csT": csT,
            }
        )
    return in_maps


def kernel(h, mask, cos, sin, wq, bq, wk, bk, wv, bv, wo, bo, **_unused):
    # mask is all-ones per the problem spec; post-softmax where(mask==0) is a no-op.
    from concourse.bass_utils import run_bass_kernel_spmd

    h = np.asarray(h, dtype=np.float32)
    S = h.shape[1]
    nc = get_nc(S)
    in_maps = make_in_maps(h, cos, sin, wq, bq, wk, bk, wv, bv, wo)
    res = run_bass_kernel_spmd(nc, in_maps, core_ids=list(range(NCORES)))
    acc = np.zeros((S, HIDDEN), dtype=np.float64)
    for r in res.results:
        acc += r["out"].astype(np.float64)
    acc /= 4096.0
    bo_eff = np.asarray(bo, np.float64) + np.asarray(wo, np.float64) @ np.asarray(
        bv, np.float64
    )
    acc += bo_eff[None, :]
    return acc[None].astype(np.float32)

